# revision 5
# baseline (speedup 1.0000x reference)
"""Trainium2 Bass kernel for nn_CoordsToNRF.

out[b, p] = atom_nc[b, p] * (AU2KCALMOLA / MAX_NRF) / ||coords[b, I[p]] - coords[b, J[p]]||^2

Strategy (pure data parallel over batch, 8 cores x 128 batches):
  - Layout: batch on partitions, pairs on the free dim ([128, 8128] per core).
  - Pair gather+subtract on the TensorEngine: per xyz component,
        D_c = CT_c.T @ S
    with S [atom, pairs] the static +1/-1 tril selection matrix. fp16 matmuls
    (1 cyc/row) with an exact THREE-term split (bit-exact fp32 reconstruction):
        C = C0 + 2^-14*C1 + 2^-14*C2
        C0 = fp16(C), C1 = fp16(2^14(C-C0)), C2 = fp16(2^14(C-C0-2^-14*C1))
    Terms 1,2 use S_s = S * 2^-14 (exact in fp16), all 3 accumulate in PSUM.
  - ScalarE squares all 3 planes in one activation with folded scale
    s = 1/sqrt(K):  r2' = r2/K.
  - VectorE (1024-wide macro ops over 2 groups): add, add,
    reciprocal_approx_fast (18-bit accurate).
  - GpSimd: final multiply by atom_nc; DMA out per macro.
"""

import sys

for _p in ("/opt/trn_rl_repo",):
    if _p not in sys.path:
        sys.path.insert(0, _p)

import numpy as np
from contextlib import ExitStack

import concourse.bass as bass
import concourse.bacc as bacc
import concourse.tile as tile
from concourse import mybir
from concourse.bass_utils import run_bass_kernel_spmd

F32 = mybir.dt.float32
F16 = mybir.dt.float16

N_ATOMS = 128
NC2 = N_ATOMS * (N_ATOMS - 1) // 2  # 8128
BATCH = 1024
N_CORES = 8
BPC = BATCH // N_CORES  # 128 batches per core

AU2KCALMOLA = 627.5095 * 0.529177
MAX_NRF = 13036.0
K_CONST = AU2KCALMOLA / MAX_NRF
SQ_SCALE = float(1.0 / np.sqrt(K_CONST))  # fold K into the square
LO_SHIFT = 2.0 ** 14

GROUP = 512  # pairs per group (one PSUM bank of fp32)
GROUPS = [(g, min(GROUP, NC2 - g)) for g in range(0, NC2, GROUP)]
MACRO = 1024  # elementwise macro block = 2 groups
MACROS = [(m, min(MACRO, NC2 - m)) for m in range(0, NC2, MACRO)]
CHUNK = 2048  # input-load chunk = 4 groups
CHUNKS = [(c, min(CHUNK, NC2 - c)) for c in range(0, NC2, CHUNK)]

_I, _J = np.tril_indices(N_ATOMS, -1)


def _build_smat_f16() -> np.ndarray:
    s = np.zeros((N_ATOMS, NC2), dtype=np.float16)
    p = np.arange(NC2)
    s[_I, p] = 1.0
    s[_J, p] = -1.0
    return s


def _split_coords3(coords32: np.ndarray):
    """[B, A*3] f32 -> three fp16 terms with C == C0 + (C1 + C2)/2^14 exactly."""
    c64 = coords32.astype(np.float64)
    c0 = c64.astype(np.float16)
    r1 = c64 - c0.astype(np.float64)
    c1 = (r1 * LO_SHIFT).astype(np.float16)
    r2_ = r1 - c1.astype(np.float64) / LO_SHIFT
    c2 = (r2_ * LO_SHIFT).astype(np.float16)
    return c0, c1, c2


def _build_program():
    nc = bacc.Bacc("TRN2", target_bir_lowering=False, debug=False)

    ch_d = [
        nc.dram_tensor(f"coords_h{t}", [BPC, N_ATOMS * 3], F16, kind="ExternalInput")
        for t in range(3)
    ]
    anc_d = nc.dram_tensor("atom_nc", [BPC, NC2], F32, kind="ExternalInput")
    smat_d = nc.dram_tensor("smat", [N_ATOMS, NC2], F16, kind="ExternalInput")
    ident_d = nc.dram_tensor("ident", [128, 128], F16, kind="ExternalInput")
    out_d = nc.dram_tensor("out", [BPC, NC2], F32, kind="ExternalOutput")

    with tile.TileContext(nc) as tc, ExitStack() as ctx:
        const = ctx.enter_context(tc.tile_pool(name="const", bufs=1))
        work = ctx.enter_context(tc.tile_pool(name="work", bufs=3))
        outp = ctx.enter_context(tc.tile_pool(name="outp", bufs=3))
        ps_t = ctx.enter_context(tc.tile_pool(name="ps_t", bufs=1, space="PSUM"))
        ps_d = ctx.enter_context(tc.tile_pool(name="ps_d", bufs=2, space="PSUM"))

        # ---- constants / inputs ----
        ident_sb = const.tile([128, 128], F16)
        nc.sync.dma_start(ident_sb[:], ident_d[:, :])

        ch_sb = []
        for t in range(3):
            cs = const.tile([BPC, N_ATOMS, 3], F16, tag=f"ch{t}")
            nc.sync.dma_start(cs[:], ch_d[t][:, :].rearrange("b (a c) -> b a c", c=3))
            ch_sb.append(cs)

        # chunked loads so early groups don't wait on the whole 8k columns
        smat_sb, ss_sb, anc_sb = [], [], []
        for ci, (c0, cw) in enumerate(CHUNKS):
            st = const.tile([N_ATOMS, cw], F16, tag=f"smat{ci}")
            nc.sync.dma_start(st[:], smat_d[:, c0:c0 + cw])
            smat_sb.append(st)
            at = const.tile([BPC, cw], F32, tag=f"anc{ci}")
            nc.sync.dma_start(at[:], anc_d[:, c0:c0 + cw])
            anc_sb.append(at)
            # S_s = S * 2^-14 (exact in fp16; +-2^-14 is the min normal)
            sl = const.tile([N_ATOMS, cw], F16, tag=f"ss{ci}")
            nc.vector.tensor_scalar_mul(sl[:], st[:], 1.0 / LO_SHIFT)
            ss_sb.append(sl)

        # ---- coords transposes: CT[t] [atom, 3, batch] fp16 ----
        ct_sb = []
        for t in range(3):
            dst = const.tile([N_ATOMS, 3, BPC], F16, tag=f"ct{t}")
            for c in range(3):
                t_ps = ps_t.tile([128, 128], F16)
                nc.tensor.transpose(t_ps[:], ch_sb[t][:, :, c], ident_sb[:])
                nc.scalar.copy(dst[:, c, :], t_ps[:])
            ct_sb.append(dst)

        # ---- main loop: group-pairs share LDWEIGHTS (2 MMs per stationary) ----
        n_g = len(GROUPS)
        for mi, (ms, mw) in enumerate(MACROS):
            gpair = [(gs, fd) for gs, fd in GROUPS if ms <= gs < ms + mw]
            d_tiles = []
            for gs, fd in gpair:
                d_t = ps_d.tile([128, 3, GROUP], F32, tag="d")
                d_tiles.append(d_t)
            for c in range(3):
                for t in range(3):
                    rhs_pool = smat_sb if t == 0 else ss_sb
                    for gi, (gs, fd) in enumerate(gpair):
                        ci, off = gs // CHUNK, gs % CHUNK
                        nc.tensor.matmul(
                            d_tiles[gi][:, c, :fd], ct_sb[t][:, c, :],
                            rhs_pool[ci][:, off:off + fd],
                            start=(t == 0), stop=(t == 2),
                            skip_group_check=True,
                        )
            # squares of all 3 planes in one activation per group (PSUM->SBUF)
            sq = work.tile([128, 3, MACRO], F32, tag="sq")
            for gi, (gs, fd) in enumerate(gpair):
                o0 = gi * GROUP
                nc.scalar.activation(
                    sq[:, :, o0:o0 + fd], d_tiles[gi][:, :, :fd],
                    mybir.ActivationFunctionType.Square,
                    bias=0.0, scale=SQ_SCALE,
                )
            # macro-wide vector ops (contiguous because groups are adjacent)
            t01 = work.tile([128, MACRO], F32, tag="t01")
            nc.vector.tensor_add(t01[:, :mw], sq[:, 0, :mw], sq[:, 1, :mw])
            r2m = work.tile([128, MACRO], F32, tag="r2m")
            nc.vector.tensor_add(r2m[:, :mw], t01[:, :mw], sq[:, 2, :mw])
            inv = work.tile([128, MACRO], F32, tag="inv")
            nc.vector.reciprocal_approx_fast(inv[:, :mw], r2m[:, :mw])
            o = outp.tile([128, MACRO], F32)
            ci, off = ms // CHUNK, ms % CHUNK
            nc.gpsimd.tensor_mul(o[:, :mw], inv[:, :mw], anc_sb[ci][:, off:off + mw])
            nc.sync.dma_start(out_d[:, ms:ms + mw], o[:, :mw])

    nc.compile()
    return nc


_CACHED = None


def _get_program():
    global _CACHED
    if _CACHED is None:
        _CACHED = _build_program()
    return _CACHED


def kernel(coords, atom_nc, _trace=False, _trace_kwargs=None):
    coords = np.ascontiguousarray(np.asarray(coords, dtype=np.float32))
    atom_nc = np.ascontiguousarray(np.asarray(atom_nc, dtype=np.float32))
    assert coords.shape == (BATCH, N_ATOMS, 3)
    assert atom_nc.shape == (BATCH, NC2)

    nc = _get_program()
    smat = _build_smat_f16()
    ident = np.eye(128, dtype=np.float16)
    c0, c1, c2 = _split_coords3(coords.reshape(BATCH, N_ATOMS * 3))

    in_maps = []
    for core in range(N_CORES):
        b0 = core * BPC
        in_maps.append({
            "coords_h0": c0[b0:b0 + BPC],
            "coords_h1": c1[b0:b0 + BPC],
            "coords_h2": c2[b0:b0 + BPC],
            "atom_nc": atom_nc[b0:b0 + BPC],
            "smat": smat,
            "ident": ident,
        })

    kw = {}
    if _trace:
        kw["trace"] = True
        kw.update(_trace_kwargs or {})
    res = run_bass_kernel_spmd(nc, in_maps, core_ids=list(range(N_CORES)), **kw)
    out = np.concatenate([r["out"] for r in res.results], axis=0)
    if _trace:
        return out, res
    return out


if __name__ == "__main__":
    rng = np.random.default_rng(0)
    coords = (rng.standard_normal((BATCH, N_ATOMS, 3)) * 5.0).astype(np.float32)
    atom_nc = rng.uniform(1.0, 50.0, (BATCH, NC2)).astype(np.float32)
    out = kernel(coords, atom_nc)
    print(out.shape, out.dtype)


# revision 8
# speedup vs baseline: 1.0489x; 1.0489x over previous
"""Trainium2 Bass kernel for nn_CoordsToNRF.

out[b, p] = atom_nc[b, p] * (AU2KCALMOLA / MAX_NRF) / ||coords[b, I[p]] - coords[b, J[p]]||^2

Strategy (pure data parallel over batch, 8 cores x 128 batches):
  - Layout: batch on partitions, pairs on the free dim ([128, 8128] per core).
  - Pair gather+subtract on the TensorEngine: per xyz component,
        D_c = CT_c.T @ S
    with S [atom, pairs] the static +1/-1 tril selection matrix. fp16 matmuls
    (1 cyc/row) with an exact THREE-term split (bit-exact fp32 reconstruction):
        C = C0 + 2^-14*C1 + 2^-14*C2
        C0 = fp16(C), C1 = fp16(2^14(C-C0)), C2 = fp16(2^14(C-C0-2^-14*C1))
    Terms 1,2 use S_s = S * 2^-14 (exact in fp16), all 3 accumulate in PSUM.
  - ScalarE squares all 3 planes in one activation with folded scale
    s = 1/sqrt(K):  r2' = r2/K.
  - VectorE (1024-wide macro ops over 2 groups): add, add,
    reciprocal_approx_fast (18-bit accurate).
  - GpSimd: final multiply by atom_nc; DMA out per macro.
"""

import sys

for _p in ("/opt/trn_rl_repo",):
    if _p not in sys.path:
        sys.path.insert(0, _p)

import numpy as np
from contextlib import ExitStack

import concourse.bass as bass
import concourse.bacc as bacc
import concourse.tile as tile
from concourse import mybir
from concourse.bass_utils import run_bass_kernel_spmd

F32 = mybir.dt.float32
F16 = mybir.dt.float16

N_ATOMS = 128
NC2 = N_ATOMS * (N_ATOMS - 1) // 2  # 8128
BATCH = 1024
N_CORES = 8
BPC = BATCH // N_CORES  # 128 batches per core

AU2KCALMOLA = 627.5095 * 0.529177
MAX_NRF = 13036.0
K_CONST = AU2KCALMOLA / MAX_NRF
SQ_SCALE = float(1.0 / np.sqrt(K_CONST))  # fold K into the square
LO_SHIFT = 2.0 ** 14

GROUP = 512  # pairs per group (one PSUM bank of fp32)
GROUPS = [(g, min(GROUP, NC2 - g)) for g in range(0, NC2, GROUP)]
MACRO = 1024  # elementwise macro block = 2 groups
MACROS = [(m, min(MACRO, NC2 - m)) for m in range(0, NC2, MACRO)]
CHUNK = 2048  # input-load chunk = 4 groups
CHUNKS = [(c, min(CHUNK, NC2 - c)) for c in range(0, NC2, CHUNK)]

_I, _J = np.tril_indices(N_ATOMS, -1)


def _build_smat_f16() -> np.ndarray:
    s = np.zeros((N_ATOMS, NC2), dtype=np.float16)
    p = np.arange(NC2)
    s[_I, p] = 1.0
    s[_J, p] = -1.0
    return s


def _split_coords3(coords32: np.ndarray):
    """[B, A*3] f32 -> three fp16 terms with C == C0 + (C1 + C2)/2^14 exactly."""
    c64 = coords32.astype(np.float64)
    c0 = c64.astype(np.float16)
    r1 = c64 - c0.astype(np.float64)
    c1 = (r1 * LO_SHIFT).astype(np.float16)
    r2_ = r1 - c1.astype(np.float64) / LO_SHIFT
    c2 = (r2_ * LO_SHIFT).astype(np.float16)
    return c0, c1, c2


def _build_program():
    nc = bacc.Bacc("TRN2", target_bir_lowering=False, debug=False)

    ch_d = [
        nc.dram_tensor(f"coords_h{t}", [BPC, N_ATOMS * 3], F16, kind="ExternalInput")
        for t in range(3)
    ]
    anc_d = nc.dram_tensor("atom_nc", [BPC, NC2], F32, kind="ExternalInput")
    smat_d = nc.dram_tensor("smat", [N_ATOMS, NC2], F16, kind="ExternalInput")
    ident_d = nc.dram_tensor("ident", [128, 128], F16, kind="ExternalInput")
    out_d = nc.dram_tensor("out", [BPC, NC2], F32, kind="ExternalOutput")

    with tile.TileContext(nc) as tc, ExitStack() as ctx:
        const = ctx.enter_context(tc.tile_pool(name="const", bufs=1))
        work = ctx.enter_context(tc.tile_pool(name="work", bufs=4))
        outp = ctx.enter_context(tc.tile_pool(name="outp", bufs=4))
        ps_t = ctx.enter_context(tc.tile_pool(name="ps_t", bufs=1, space="PSUM"))
        ps_d = ctx.enter_context(tc.tile_pool(name="ps_d", bufs=2, space="PSUM"))

        # ---- constants / inputs ----
        ident_sb = const.tile([128, 128], F16)
        nc.sync.dma_start(ident_sb[:], ident_d[:, :])

        ch_sb = []
        for t in range(3):
            cs = const.tile([BPC, N_ATOMS, 3], F16, tag=f"ch{t}")
            nc.sync.dma_start(cs[:], ch_d[t][:, :].rearrange("b (a c) -> b a c", c=3))
            ch_sb.append(cs)

        # chunked loads so early groups don't wait on the whole 8k columns.
        # smat first (gates the PE), atom_nc after (only needed by the final
        # multiply, much later in the pipeline).
        smat_sb, ss_sb, anc_sb = [], [], []
        for ci, (c0, cw) in enumerate(CHUNKS):
            st = const.tile([N_ATOMS, cw], F16, tag=f"smat{ci}")
            nc.sync.dma_start(st[:], smat_d[:, c0:c0 + cw])
            smat_sb.append(st)
            # S_s = S * 2^-14 (exact in fp16; +-2^-14 is the min normal)
            sl = const.tile([N_ATOMS, cw], F16, tag=f"ss{ci}")
            nc.vector.tensor_scalar_mul(sl[:], st[:], 1.0 / LO_SHIFT)
            ss_sb.append(sl)
        for ci, (c0, cw) in enumerate(CHUNKS):
            at = const.tile([BPC, cw], F32, tag=f"anc{ci}")
            nc.sync.dma_start(at[:], anc_d[:, c0:c0 + cw])
            anc_sb.append(at)

        # ---- coords transposes: CT[t] [atom, 3, batch] fp16 ----
        ct_sb = []
        for t in range(3):
            dst = const.tile([N_ATOMS, 3, BPC], F16, tag=f"ct{t}")
            for c in range(3):
                t_ps = ps_t.tile([128, 128], F16)
                nc.tensor.transpose(t_ps[:], ch_sb[t][:, :, c], ident_sb[:])
                nc.scalar.copy(dst[:, c, :], t_ps[:])
            ct_sb.append(dst)

        # ---- main loop: group-pairs share LDWEIGHTS (2 MMs per stationary) ----
        n_g = len(GROUPS)
        for mi, (ms, mw) in enumerate(MACROS):
            gpair = [(gs, fd) for gs, fd in GROUPS if ms <= gs < ms + mw]
            d_tiles = []
            for gs, fd in gpair:
                d_t = ps_d.tile([128, 3, GROUP], F32, tag="d")
                d_tiles.append(d_t)
            for c in range(3):
                for t in range(3):
                    rhs_pool = smat_sb if t == 0 else ss_sb
                    for gi, (gs, fd) in enumerate(gpair):
                        ci, off = gs // CHUNK, gs % CHUNK
                        nc.tensor.matmul(
                            d_tiles[gi][:, c, :fd], ct_sb[t][:, c, :],
                            rhs_pool[ci][:, off:off + fd],
                            start=(t == 0), stop=(t == 2),
                            skip_group_check=True,
                        )
            # squares of all 3 planes in one activation per group (PSUM->SBUF)
            sq = work.tile([128, 3, MACRO], F32, tag="sq")
            for gi, (gs, fd) in enumerate(gpair):
                o0 = gi * GROUP
                nc.scalar.activation(
                    sq[:, :, o0:o0 + fd], d_tiles[gi][:, :, :fd],
                    mybir.ActivationFunctionType.Square,
                    bias=0.0, scale=SQ_SCALE,
                )
            # macro-wide vector ops (contiguous because groups are adjacent)
            t01 = work.tile([128, MACRO], F32, tag="t01")
            nc.vector.tensor_add(t01[:, :mw], sq[:, 0, :mw], sq[:, 1, :mw])
            r2m = work.tile([128, MACRO], F32, tag="r2m")
            nc.vector.tensor_add(r2m[:, :mw], t01[:, :mw], sq[:, 2, :mw])
            inv = work.tile([128, MACRO], F32, tag="inv")
            nc.vector.reciprocal_approx_fast(inv[:, :mw], r2m[:, :mw])
            o = outp.tile([128, MACRO], F32)
            ci, off = ms // CHUNK, ms % CHUNK
            # alternate the final multiply between GpSimd and Vector so
            # neither engine gates downstream tile recycling
            mul_eng = nc.gpsimd if mi % 2 == 0 else nc.vector
            mul_eng.tensor_mul(o[:, :mw], inv[:, :mw], anc_sb[ci][:, off:off + mw])
            nc.sync.dma_start(out_d[:, ms:ms + mw], o[:, :mw])

    nc.compile()
    return nc


_CACHED = None


def _get_program():
    global _CACHED
    if _CACHED is None:
        _CACHED = _build_program()
    return _CACHED


def kernel(coords, atom_nc, _trace=False, _trace_kwargs=None):
    coords = np.ascontiguousarray(np.asarray(coords, dtype=np.float32))
    atom_nc = np.ascontiguousarray(np.asarray(atom_nc, dtype=np.float32))
    assert coords.shape == (BATCH, N_ATOMS, 3)
    assert atom_nc.shape == (BATCH, NC2)

    nc = _get_program()
    smat = _build_smat_f16()
    ident = np.eye(128, dtype=np.float16)
    c0, c1, c2 = _split_coords3(coords.reshape(BATCH, N_ATOMS * 3))

    in_maps = []
    for core in range(N_CORES):
        b0 = core * BPC
        in_maps.append({
            "coords_h0": c0[b0:b0 + BPC],
            "coords_h1": c1[b0:b0 + BPC],
            "coords_h2": c2[b0:b0 + BPC],
            "atom_nc": atom_nc[b0:b0 + BPC],
            "smat": smat,
            "ident": ident,
        })

    kw = {}
    if _trace:
        kw["trace"] = True
        kw.update(_trace_kwargs or {})
    res = run_bass_kernel_spmd(nc, in_maps, core_ids=list(range(N_CORES)), **kw)
    out = np.concatenate([r["out"] for r in res.results], axis=0)
    if _trace:
        return out, res
    return out


if __name__ == "__main__":
    rng = np.random.default_rng(0)
    coords = (rng.standard_normal((BATCH, N_ATOMS, 3)) * 5.0).astype(np.float32)
    atom_nc = rng.uniform(1.0, 50.0, (BATCH, NC2)).astype(np.float32)
    out = kernel(coords, atom_nc)
    print(out.shape, out.dtype)


# revision 10
# speedup vs baseline: 1.1335x; 1.0806x over previous
"""Trainium2 Bass kernel for nn_CoordsToNRF.

out[b, p] = atom_nc[b, p] * (AU2KCALMOLA / MAX_NRF) / ||coords[b, I[p]] - coords[b, J[p]]||^2

Strategy (pure data parallel over batch, 8 cores x 128 batches):
  - Layout: batch on partitions, pairs on the free dim ([128, 8128] per core).
  - Pair gather+subtract on the TensorEngine: per xyz component,
        D_c = CT_c.T @ S
    with S [atom, pairs] the static +1/-1 tril selection matrix. fp16 matmuls
    (1 cyc/row) with an exact THREE-term split (bit-exact fp32 reconstruction):
        C = C0 + 2^-14*C1 + 2^-14*C2
        C0 = fp16(C), C1 = fp16(2^14(C-C0)), C2 = fp16(2^14(C-C0-2^-14*C1))
    Terms 1,2 use S_s = S * 2^-14 (exact in fp16), all 3 accumulate in PSUM.
  - ScalarE squares all 3 planes in one activation with folded scale
    s = 1/sqrt(K):  r2' = r2/K.
  - VectorE (1024-wide macro ops over 2 groups): add, add,
    reciprocal_approx_fast (18-bit accurate).
  - GpSimd: final multiply by atom_nc; DMA out per macro.
"""

import sys

for _p in ("/opt/trn_rl_repo",):
    if _p not in sys.path:
        sys.path.insert(0, _p)

import numpy as np
from contextlib import ExitStack

import concourse.bass as bass
import concourse.bacc as bacc
import concourse.tile as tile
from concourse import mybir
from concourse.bass_utils import run_bass_kernel_spmd

F32 = mybir.dt.float32
F16 = mybir.dt.float16

N_ATOMS = 128
NC2 = N_ATOMS * (N_ATOMS - 1) // 2  # 8128
BATCH = 1024
N_CORES = 8
BPC = BATCH // N_CORES  # 128 batches per core

AU2KCALMOLA = 627.5095 * 0.529177
MAX_NRF = 13036.0
K_CONST = AU2KCALMOLA / MAX_NRF
SQ_SCALE = float(1.0 / np.sqrt(K_CONST))  # fold K into the square
LO_SHIFT = 2.0 ** 14

GROUP = 512  # pairs per group (one PSUM bank of fp32)
GROUPS = [(g, min(GROUP, NC2 - g)) for g in range(0, NC2, GROUP)]
MACRO = 1024  # elementwise macro block = 2 groups
MACROS = [(m, min(MACRO, NC2 - m)) for m in range(0, NC2, MACRO)]
CHUNK = 2048  # input-load chunk = 4 groups
CHUNKS = [(c, min(CHUNK, NC2 - c)) for c in range(0, NC2, CHUNK)]

_I, _J = np.tril_indices(N_ATOMS, -1)


def _build_smat_f16() -> np.ndarray:
    s = np.zeros((N_ATOMS, NC2), dtype=np.float16)
    p = np.arange(NC2)
    s[_I, p] = 1.0
    s[_J, p] = -1.0
    return s


def _split_coords3(coords32: np.ndarray):
    """[B, A*3] f32 -> three fp16 terms with C == C0 + (C1 + C2)/2^14 exactly."""
    c64 = coords32.astype(np.float64)
    c0 = c64.astype(np.float16)
    r1 = c64 - c0.astype(np.float64)
    c1 = (r1 * LO_SHIFT).astype(np.float16)
    r2_ = r1 - c1.astype(np.float64) / LO_SHIFT
    c2 = (r2_ * LO_SHIFT).astype(np.float16)
    return c0, c1, c2


def _build_program():
    nc = bacc.Bacc("TRN2", target_bir_lowering=False, debug=False)

    ch_d = [
        nc.dram_tensor(f"coords_h{t}", [BPC, N_ATOMS * 3], F16, kind="ExternalInput")
        for t in range(3)
    ]
    anc_d = nc.dram_tensor("atom_nc", [BPC, NC2], F32, kind="ExternalInput")
    smat_d = nc.dram_tensor("smat", [N_ATOMS, NC2], F16, kind="ExternalInput")
    ident_d = nc.dram_tensor("ident", [128, 128], F16, kind="ExternalInput")
    out_d = nc.dram_tensor("out", [BPC, NC2], F32, kind="ExternalOutput")

    with tile.TileContext(nc) as tc, ExitStack() as ctx:
        const = ctx.enter_context(tc.tile_pool(name="const", bufs=1))
        work = ctx.enter_context(tc.tile_pool(name="work", bufs=4))
        outp = ctx.enter_context(tc.tile_pool(name="outp", bufs=4))
        ps_t = ctx.enter_context(tc.tile_pool(name="ps_t", bufs=1, space="PSUM"))
        ps_d = ctx.enter_context(tc.tile_pool(name="ps_d", bufs=7, space="PSUM"))

        # ---- constants / inputs ----
        ident_sb = const.tile([128, 128], F16)
        nc.sync.dma_start(ident_sb[:], ident_d[:, :])

        ch_sb = []
        for t in range(3):
            cs = const.tile([BPC, N_ATOMS, 3], F16, tag=f"ch{t}")
            nc.sync.dma_start(cs[:], ch_d[t][:, :].rearrange("b (a c) -> b a c", c=3))
            ch_sb.append(cs)

        # chunked loads so early groups don't wait on the whole 8k columns.
        # smat first (gates the PE), atom_nc after (only needed by the final
        # multiply, much later in the pipeline).
        smat_sb, ss_sb, anc_sb = [], [], []
        for ci, (c0, cw) in enumerate(CHUNKS):
            st = const.tile([N_ATOMS, cw], F16, tag=f"smat{ci}")
            nc.sync.dma_start(st[:], smat_d[:, c0:c0 + cw])
            smat_sb.append(st)
            # S_s = S * 2^-14 (exact in fp16; +-2^-14 is the min normal)
            sl = const.tile([N_ATOMS, cw], F16, tag=f"ss{ci}")
            nc.vector.tensor_scalar_mul(sl[:], st[:], 1.0 / LO_SHIFT)
            ss_sb.append(sl)
        for ci, (c0, cw) in enumerate(CHUNKS):
            at = const.tile([BPC, cw], F32, tag=f"anc{ci}")
            nc.sync.dma_start(at[:], anc_d[:, c0:c0 + cw])
            anc_sb.append(at)

        # ---- coords transposes: CT[t] [atom, 3, batch] fp16 ----
        ct_sb = []
        for t in range(3):
            dst = const.tile([N_ATOMS, 3, BPC], F16, tag=f"ct{t}")
            for c in range(3):
                t_ps = ps_t.tile([128, 128], F16)
                nc.tensor.transpose(t_ps[:], ch_sb[t][:, :, c], ident_sb[:])
                nc.scalar.copy(dst[:, c, :], t_ps[:])
            ct_sb.append(dst)

        # ---- main loop: group-pairs share LDWEIGHTS (2 MMs per stationary) ----
        n_g = len(GROUPS)
        for mi, (ms, mw) in enumerate(MACROS):
            gpair = [(gs, fd) for gs, fd in GROUPS if ms <= gs < ms + mw]
            # one PSUM bank per (group, plane): PE can run ~2 groups ahead of
            # the scalar squares instead of stalling on whole-macro tiles
            d_tiles = []
            for gi in range(len(gpair)):
                d_row = []
                for c in range(3):
                    d_t = ps_d.tile([128, GROUP], F32, tag="d")
                    d_row.append(d_t)
                d_tiles.append(d_row)
            sq = work.tile([128, 3, MACRO], F32, tag="sq")
            for c in range(3):
                for t in range(3):
                    rhs_pool = smat_sb if t == 0 else ss_sb
                    for gi, (gs, fd) in enumerate(gpair):
                        ci, off = gs // CHUNK, gs % CHUNK
                        nc.tensor.matmul(
                            d_tiles[gi][c][:, :fd], ct_sb[t][:, c, :],
                            rhs_pool[ci][:, off:off + fd],
                            start=(t == 0), stop=(t == 2),
                            skip_group_check=True,
                        )
                # square each plane as soon as its accumulation finishes
                for gi, (gs, fd) in enumerate(gpair):
                    o0 = gi * GROUP
                    nc.scalar.activation(
                        sq[:, c, o0:o0 + fd], d_tiles[gi][c][:, :fd],
                        mybir.ActivationFunctionType.Square,
                        bias=0.0, scale=SQ_SCALE,
                    )
            # macro-wide vector ops (contiguous because groups are adjacent)
            t01 = work.tile([128, MACRO], F32, tag="t01")
            nc.vector.tensor_add(t01[:, :mw], sq[:, 0, :mw], sq[:, 1, :mw])
            r2m = work.tile([128, MACRO], F32, tag="r2m")
            nc.vector.tensor_add(r2m[:, :mw], t01[:, :mw], sq[:, 2, :mw])
            inv = work.tile([128, MACRO], F32, tag="inv")
            nc.vector.reciprocal_approx_fast(inv[:, :mw], r2m[:, :mw])
            o = outp.tile([128, MACRO], F32)
            ci, off = ms // CHUNK, ms % CHUNK
            # alternate the final multiply between GpSimd and Vector so
            # neither engine gates downstream tile recycling
            mul_eng = nc.gpsimd if mi % 2 == 0 else nc.vector
            mul_eng.tensor_mul(o[:, :mw], inv[:, :mw], anc_sb[ci][:, off:off + mw])
            nc.sync.dma_start(out_d[:, ms:ms + mw], o[:, :mw])

    nc.compile()
    return nc


_CACHED = None


def _get_program():
    global _CACHED
    if _CACHED is None:
        _CACHED = _build_program()
    return _CACHED


def kernel(coords, atom_nc, _trace=False, _trace_kwargs=None):
    coords = np.ascontiguousarray(np.asarray(coords, dtype=np.float32))
    atom_nc = np.ascontiguousarray(np.asarray(atom_nc, dtype=np.float32))
    assert coords.shape == (BATCH, N_ATOMS, 3)
    assert atom_nc.shape == (BATCH, NC2)

    nc = _get_program()
    smat = _build_smat_f16()
    ident = np.eye(128, dtype=np.float16)
    c0, c1, c2 = _split_coords3(coords.reshape(BATCH, N_ATOMS * 3))

    in_maps = []
    for core in range(N_CORES):
        b0 = core * BPC
        in_maps.append({
            "coords_h0": c0[b0:b0 + BPC],
            "coords_h1": c1[b0:b0 + BPC],
            "coords_h2": c2[b0:b0 + BPC],
            "atom_nc": atom_nc[b0:b0 + BPC],
            "smat": smat,
            "ident": ident,
        })

    kw = {}
    if _trace:
        kw["trace"] = True
        kw.update(_trace_kwargs or {})
    res = run_bass_kernel_spmd(nc, in_maps, core_ids=list(range(N_CORES)), **kw)
    out = np.concatenate([r["out"] for r in res.results], axis=0)
    if _trace:
        return out, res
    return out


if __name__ == "__main__":
    rng = np.random.default_rng(0)
    coords = (rng.standard_normal((BATCH, N_ATOMS, 3)) * 5.0).astype(np.float32)
    atom_nc = rng.uniform(1.0, 50.0, (BATCH, NC2)).astype(np.float32)
    out = kernel(coords, atom_nc)
    print(out.shape, out.dtype)
